# revision 5
# baseline (speedup 1.0000x reference)
"""Trainium2 Bass kernel for nn_CabbageHeadRefinementLoss — pruned redesign.

Self-contained: accepts FULL inputs, shards across 8 NeuronCores internally,
returns the FULL (scalar) output.

Strategy (v2 — x-sort pruned ball query):
  - Boundary points (~3250 of 8192) are compacted AND SORTED BY X on host,
    padded to NBP=3584.  Each core owns an i-slab of 896 sorted ranks; only
    j's within PAD=256 ranks of the slab can be within R=0.05 in x (host
    asserts the true rank window <= PAD), so each core processes an 11-chunk
    j-window of 1408 ranks instead of all 3584: 4480 matmul columns/core
    instead of 25088 (5.6x less PE work).
  - Per j-chunk k (128 j's), i-columns C_k = [128k-512, 128k+128) ∩ [0,896):
    mm1 (bf16 hi/lo split, K=11) -> d2 PSUM; threshold alternates DVE is_gt
    (even k) / ACT Sign with half-weights (odd k, corrected on host); mm2
    accumulates [1, p2, p2^2] per 128-col block with exact start/stop flags.
  - No PE warm-up: total matmul work (~9k cycles) is below the HAM unthrottle
    threshold, so the loop targets the steady 1.2 GHz p-state.
  - O(N) terms: refinement/consistency/target-counts sharded in quarters
    across the 4 cores of a sample (bf16 inputs); pred-head mask computed on
    host (must be exact); moments + center + distance pass replicated.
    sqrt(s) computed as exp(0.5*ln(s)) so only ONE ACT table set (id 6:
    exp/ln/sign/copy) is ever loaded.
  - All inputs packed into 3 per-partition-contiguous DMA blobs; outputs are
    row-contiguous (no [128,1]-shaped DMAs -> no completion-counter straggler).
  - Host combines: per-row variance, 3x3 eigendecomposition, gates, total.
"""

import numpy as np

try:
    import concourse.bass as bass
except ImportError:  # fallback for environments without NIX_PYTHONPATH
    import sys
    sys.path.insert(0, "/opt/trn_rl_repo")
    import concourse.bass as bass

import concourse.mybir as mybir
import concourse.tile as tile
from concourse import bacc
from concourse.bass_utils import run_bass_kernel_spmd

F32 = mybir.dt.float32
BF16 = mybir.dt.bfloat16
ALU = mybir.AluOpType
ACTF = mybir.ActivationFunctionType

B, N, C = 2, 8192, 3
R2 = np.float32(0.05) * np.float32(0.05)
W_REF, W_CON, W_BND = 0.3, 0.2, 2.0
W_SHP, W_SMO, W_SIZ, W_CNN = 0.5, 0.3, 0.8, 0.6

NBP = 3584
SLAB = 896          # i-ranks per core
PAD = 256           # j rank-window halo (host asserts data fits)
JW = SLAB + 2 * PAD  # 1408 j-window
NCH = JW // 128      # 11 j-chunks
NBLK = SLAB // 128   # 7 acc column blocks
QN = N // 4          # 2048 per-core quarter for sharded O(N) terms
FQ = QN // 128       # 16
FN = N // 128        # 64
NCORES = 8

SIGN_CHUNKS = (1, 3, 5, 7, 9)   # chunks thresholded via ACT Sign (half-weights)

# sums row layout (host side):
#   [0]=n  [1:4]=Smx  [4:10]=M2(xx,yy,zz,xy,xz,yz)          <- ones-matmul #1
#   [10]=nllw_q [11:14]=cons_c [14]=ngt_q [15]=bm_sum_q
#   [16]=Smd_q [17]=Smd2_q [18:33]=vodd(k in SIGN_CHUNKS)   <- ones-matmul #2
#   [33]=maxd_q

_NC_CACHE = None


def _chunk_cols(k):
    return max(0, 128 * k - 2 * PAD), min(SLAB, 128 * k + 128)


def _blk_range(k):
    return max(0, k - 4), min(NBLK - 1, k)


def _build_nc():
    nc = bacc.Bacc("TRN2", target_bir_lowering=False, debug=False,
                   enable_asserts=False)

    rq_d = nc.dram_tensor("rq", [11, JW + SLAB], BF16, kind="ExternalInput").ap()
    hb_d = nc.dram_tensor("hb", [128, 512], BF16, kind="ExternalInput").ap()
    acc_d = nc.dram_tensor("acc", [3, SLAB], F32, kind="ExternalOutput").ap()
    sums_d = nc.dram_tensor("sums", [1, 64], F32, kind="ExternalOutput").ap()

    thr = float(-R2 / 2.0)

    with tile.TileContext(nc) as tc:
        with (
            tc.tile_pool(name="const", bufs=1) as const,
            tc.tile_pool(name="work", bufs=8) as work,
            tc.tile_pool(name="tp", bufs=4) as tp,
            tc.tile_pool(name="psD", bufs=2, space="PSUM") as psD,
            tc.tile_pool(name="psS", bufs=1, space="PSUM") as psS,
            tc.tile_pool(name="psA", bufs=1, space="PSUM") as psA,
        ):
            # Load ACT set 6 (exp+ln+sign+copy) once, up front: the greedy
            # table-load pass would otherwise thrash between sets 0 and 5
            # (1.28us per reload on the Scalar critical path).
            _li = mybir.InstLoadActFuncSet(
                name=nc.get_next_instruction_name(), ins=[], outs=[],
                act_func_set_id=6)
            nc.scalar.add_instruction(_li)

            # ---------- input DMAs ----------
            RQ = const.tile([11, JW + SLAB], BF16)
            nc.sync.dma_start(RQ[:], rq_d[:])
            H = const.tile([128, 512], BF16)
            nc.gpsimd.dma_start(H[:], hb_d[:])

            RB = RQ[:, 0:JW]
            QB = RQ[:, JW:JW + SLAB]
            lbw = H[:, 448:481].rearrange("p (k c) -> p k c", c=3)
            with tc.high_priority():
                mh_t = const.tile([128, 11], F32)
                nc.vector.tensor_add(mh_t[:], H[:, 481:492], H[:, 492:503])
                biasj_t = const.tile([128, 11], F32)
                nc.vector.tensor_scalar(biasj_t[:], mh_t[:], float(R2) / 2.0,
                                        None, op0=ALU.add)
            mh = mh_t
            biasj = biasj_t
            lgq = H[:, 0:48].rearrange("p (c f) -> p c f", c=3)
            loq = H[:, 48:96].rearrange("p (c f) -> p c f", c=3)
            tgq = H[:, 96:112]
            bmq = H[:, 112:128]
            m_f = H[:, 128:192]
            ptT = H[:, 192:384].rearrange("p (c f) -> p c f", c=3)
            m_q = H[:, 384:400]
            ptq = H[:, 400:448].rearrange("p (c f) -> p c f", c=3)

            stA = const.tile([128, 10], F32)   # n, Smx, M2
            st2 = const.tile([128, 23], F32)   # nllw, cons, ge2, bm_sum, Smd,
            junk = const.tile([128, FN], F32)  # Smd2, vodd
            junk2 = const.tile([128, FN], F32)

            acc_ps = psA.tile([3, SLAB], F32, tag="accp", name="acc_ps")
            with tc.high_priority():
                nc.vector.memset(acc_ps[:], 0.0)

            def d2t(k):
                lo, hi = _chunk_cols(k)
                w = hi - lo
                lhsT = RB[:, 128 * k:128 * (k + 1)]
                with tc.high_priority():
                    d2 = psD.tile([128, 640], F32, tag="d2", name=f"d2_{k}")
                    for c0 in range(0, w, 512):
                        c1 = min(c0 + 512, w)
                        nc.tensor.matmul(d2[:, c0:c1], lhsT,
                                         QB[:, lo + c0:lo + c1],
                                         start=True, stop=True)
                    T = tp.tile([128, 640], BF16, tag="T", name=f"T_{k}")
                    if k not in SIGN_CHUNKS:
                        nc.vector.tensor_scalar(T[:, 0:w], d2[:, 0:w],
                                                mh[:, k:k + 1], thr,
                                                op0=ALU.add, op1=ALU.is_gt)
                    else:
                        nc.scalar.activation(T[:, 0:w], d2[:, 0:w], ACTF.Sign,
                                             bias=biasj[:, k:k + 1], scale=1.0)
                return T

            def accm(k, T):
                lo, hi = _chunk_cols(k)
                w = hi - lo
                V = Vh if k in SIGN_CHUNKS else Vb
                with tc.high_priority():
                    # accumulating matmul (PSUM pre-zeroed by the memset)
                    for c0 in range(0, w, 512):
                        c1 = min(c0 + 512, w)
                        nc.tensor.matmul(acc_ps[:, lo + c0:lo + c1],
                                         V[:, k, :], T[:, c0:c1],
                                         start=False, stop=False,
                                         skip_group_check=True)

            def chunk(k):
                accm(k, d2t(k))

            # ---------- chunks 0-1: thresholds first, V build, then mm2 ----------
            T0 = d2t(0)
            T1 = d2t(1)

            # ---------- boundary p2 / V (dep: H) ----------
            with tc.high_priority():
                ELB = work.tile([128, 11, 3], F32)
                nc.scalar.activation(ELB[:], lbw, ACTF.Exp)
                sb = work.tile([128, 11], F32)
                nc.vector.tensor_add(sb[:], ELB[:, :, 0], ELB[:, :, 1])
                sb2 = work.tile([128, 11], F32)
                nc.vector.tensor_add(sb2[:], sb[:], ELB[:, :, 2])
                rb = work.tile([128, 11], F32)
                nc.vector.reciprocal(rb[:], sb2[:])
                p2 = work.tile([128, 11], F32)
                nc.vector.tensor_mul(p2[:], ELB[:, :, 2], rb[:])
                Vb = const.tile([128, 11, 3], BF16)
                nc.vector.memset(Vb[:, :, 0:1], 1.0)
                nc.vector.tensor_copy(Vb[:, :, 1], p2[:])
                nc.vector.tensor_mul(Vb[:, :, 2], p2[:], p2[:])
                Vh = const.tile([128, 11, 3], BF16)
                nc.vector.tensor_scalar(Vh[:], Vb[:], 0.5, None, op0=ALU.mult)

            accm(0, T0)
            accm(1, T1)


            # ---------- O(N) stage A: count + moments (dep: H) ----------
            with tc.tile_wait_until(0.004):
                nc.vector.tensor_reduce(stA[:, 0:1], m_f,
                                        axis=mybir.AxisListType.X, op=ALU.add)
                mx = []
                for c in range(3):
                    mxc = const.tile([128, FN], F32, tag=f"mx{c}", name=f"mx{c}")
                    nc.vector.scalar_tensor_tensor(
                        out=mxc[:], in0=m_f, scalar=0.0, in1=ptT[:, c, :],
                        op0=ALU.add, op1=ALU.mult, accum_out=stA[:, 1 + c:2 + c])
                    mx.append(mxc)
                pairs = [(0, 0), (1, 1), (2, 2), (0, 1), (0, 2), (1, 2)]
                for kk, (a, bb) in enumerate(pairs):
                    nc.vector.scalar_tensor_tensor(
                        out=junk2[:], in0=mx[a][:], scalar=0.0, in1=ptT[:, bb, :],
                        op0=ALU.add, op1=ALU.mult, accum_out=stA[:, 4 + kk:5 + kk])

            chunk(2)
            chunk(3)

            # ---------- center chain ----------
            ones1 = const.tile([128, 1], F32)
            nc.vector.memset(ones1[:], 1.0)
            sums1 = psS.tile([1, 10], F32, tag="s1", name="sums1")
            nc.tensor.matmul(sums1[:], ones1[:], stA[:], start=True, stop=True)
            nz = work.tile([1, 1], F32)
            nc.vector.tensor_scalar(nz[:], sums1[0:1, 0:1], 1.0, None, op0=ALU.max)
            rcp = work.tile([1, 1], F32)
            nc.vector.reciprocal(rcp[:], nz[:])
            cen = work.tile([1, 3], F32)
            nc.vector.tensor_scalar(cen[:], sums1[0:1, 1:4], rcp[:], None,
                                    op0=ALU.mult)
            sums_sb = const.tile([1, 64], F32)
            nc.vector.tensor_copy(sums_sb[:, 0:10], sums1[:])
            ones2 = const.tile([1, 128], F32)
            nc.vector.memset(ones2[:], 1.0)
            cbp = psS.tile([128, 3], F32, tag="s1", name="cbp")
            nc.tensor.matmul(cbp[:], ones2[:], cen[:], start=True, stop=True)
            cb = const.tile([128, 3], F32)
            nc.vector.tensor_copy(cb[:], cbp[:])

            chunk(4)

            # ---------- O(N) stage B: softmax / consistency / nll (quarter) ----------
            tc.tile_set_cur_wait(0.005)
            EL = work.tile([128, 3, FQ], F32)
            nc.scalar.activation(EL[:], lgq, ACTF.Exp)
            sl = work.tile([128, FQ], F32)
            nc.vector.tensor_add(sl[:], EL[:, 0, :], EL[:, 1, :])
            sl2 = work.tile([128, FQ], F32)
            nc.vector.tensor_add(sl2[:], sl[:], EL[:, 2, :])
            rl = work.tile([128, FQ], F32)
            nc.vector.reciprocal(rl[:], sl2[:])
            EO = work.tile([128, 3, FQ], F32)
            nc.scalar.activation(EO[:], loq, ACTF.Exp)
            so = work.tile([128, FQ], F32)
            nc.gpsimd.tensor_add(so[:], EO[:, 0, :], EO[:, 1, :])
            so2 = work.tile([128, FQ], F32)
            nc.gpsimd.tensor_add(so2[:], so[:], EO[:, 2, :])
            ro = work.tile([128, FQ], F32)
            nc.vector.reciprocal(ro[:], so2[:])
            lnS = work.tile([128, FQ], F32)
            nc.scalar.activation(lnS[:], sl2[:], ACTF.Ln)

            tc.tile_set_cur_wait(0)
            chunk(5)
            tc.tile_set_cur_wait(0.005)

            for c in range(3):
                pc = work.tile([128, FQ], F32, tag="pc", name="pc")
                nc.vector.tensor_mul(pc[:], EL[:, c, :], rl[:])
                qc = work.tile([128, FQ], F32, tag="qc", name="qc")
                nc.gpsimd.tensor_mul(qc[:], EO[:, c, :], ro[:])
                dc = work.tile([128, FQ], F32, tag="dc", name="dc")
                nc.gpsimd.tensor_sub(dc[:], pc[:], qc[:])
                nc.vector.scalar_tensor_tensor(
                    out=junk2[:, 0:FQ], in0=dc[:], scalar=0.0, in1=dc[:],
                    op0=ALU.add, op1=ALU.mult, accum_out=st2[:, 1 + c:2 + c])

            lt = None
            for c in range(3):
                mc = work.tile([128, FQ], F32, tag=f"mc{c}", name=f"mc{c}")
                nc.vector.tensor_scalar(mc[:], tgq, float(c), None,
                                        op0=ALU.is_equal)
                lm = work.tile([128, FQ], F32, tag="lm", name="lm")
                nc.gpsimd.tensor_mul(lm[:], lgq[:, c, :], mc[:])
                if lt is None:
                    lt = lm
                else:
                    lt2 = work.tile([128, FQ], F32, tag="lt2", name="lt2")
                    nc.gpsimd.tensor_add(lt2[:], lt[:], lm[:])
                    lt = lt2
                if c == 2:
                    nc.vector.tensor_reduce(st2[:, 4:5], mc[:],
                                            axis=mybir.AxisListType.X, op=ALU.add)
            nll = work.tile([128, FQ], F32)
            nc.vector.tensor_sub(nll[:], lnS[:], lt[:])
            nc.vector.scalar_tensor_tensor(
                out=junk2[:, 0:FQ], in0=bmq, scalar=1.0, in1=nll[:],
                op0=ALU.add, op1=ALU.mult, accum_out=st2[:, 0:1])
            nc.vector.tensor_reduce(st2[:, 5:6], bmq, axis=mybir.AxisListType.X,
                                    op=ALU.add)
            tc.tile_set_cur_wait(0)

            chunk(6)

            # ---------- distance pass (replicated full sample) ----------
            dx = work.tile([128, FQ], F32)
            nc.vector.tensor_scalar(dx[:], ptq[:, 0, :], cb[:, 0:1], None,
                                    op0=ALU.subtract)
            dy = work.tile([128, FQ], F32)
            nc.vector.tensor_scalar(dy[:], ptq[:, 1, :], cb[:, 1:2], None,
                                    op0=ALU.subtract)
            dz = work.tile([128, FQ], F32)
            nc.vector.tensor_scalar(dz[:], ptq[:, 2, :], cb[:, 2:3], None,
                                    op0=ALU.subtract)
            s0 = work.tile([128, FQ], F32)
            nc.gpsimd.tensor_mul(s0[:], dx[:], dx[:])
            s1t = work.tile([128, FQ], F32)
            nc.vector.tensor_mul(s1t[:], dy[:], dy[:])
            s2t = work.tile([128, FQ], F32)
            nc.gpsimd.tensor_add(s2t[:], s0[:], s1t[:])
            s3t = work.tile([128, FQ], F32)
            nc.gpsimd.tensor_mul(s3t[:], dz[:], dz[:])
            s4t = work.tile([128, FQ], F32)
            nc.vector.tensor_add(s4t[:], s2t[:], s3t[:])
            eps12 = const.tile([128, 1], F32)
            nc.vector.memset(eps12[:], 1e-12)
            ls = work.tile([128, FQ], F32)
            nc.scalar.activation(ls[:], s4t[:], ACTF.Ln, bias=eps12[:, 0:1])
            dd = work.tile([128, FQ], F32)
            nc.scalar.activation(dd[:], ls[:], ACTF.Exp, bias=0.0, scale=0.5)
            md = work.tile([128, FQ], F32)
            nc.gpsimd.tensor_mul(md[:], m_q, dd[:])
            nc.vector.tensor_reduce(st2[:, 6:7], md[:], axis=mybir.AxisListType.X,
                                    op=ALU.add)
            nc.vector.scalar_tensor_tensor(
                out=junk2[:, 0:FQ], in0=md[:], scalar=0.0, in1=dd[:],
                op0=ALU.add, op1=ALU.mult, accum_out=st2[:, 7:8])
            maxsc = work.tile([1, 1], F32)
            nc.gpsimd.tensor_reduce(maxsc[:], md[:], axis=mybir.AxisListType.XYZWC,
                                    op=ALU.max)
            nc.vector.tensor_copy(sums_sb[:, 33:34], maxsc[:])

            chunk(7)

            # vodd copies for host Sign correction
            for i, ko in enumerate(SIGN_CHUNKS):
                nc.gpsimd.tensor_copy(st2[:, 8 + 3 * i:11 + 3 * i], Vb[:, ko, :])

            chunk(8)

            # acc cols [0:640] complete after chunk 8 (contributors k<=8)
            acc_sb = const.tile([3, SLAB], F32)
            nc.scalar.copy(acc_sb[:, 0:640], acc_ps[:, 0:640])
            nc.sync.dma_start(acc_d[:, 0:640], acc_sb[:, 0:640])

            chunk(9)

            sums2 = psS.tile([1, 23], F32, tag="s1", name="sums2")
            nc.tensor.matmul(sums2[:], ones1[:], st2[:], start=True, stop=True)
            nc.vector.tensor_copy(sums_sb[:, 10:33], sums2[:])
            nc.sync.dma_start(sums_d[:], sums_sb[:])
            nc.vector.tensor_copy(acc_sb[:, 640:768], acc_ps[:, 640:768])

            chunk(10)

            nc.vector.tensor_copy(acc_sb[:, 768:SLAB], acc_ps[:, 768:SLAB])
            nc.sync.dma_start(acc_d[:, 640:SLAB], acc_sb[:, 640:SLAB])

    nc.compile()
    return nc


def _get_nc():
    global _NC_CACHE
    if _NC_CACHE is None:
        _NC_CACHE = _build_nc()
    return _NC_CACHE


def _prep_inputs(logits, original_logits, head_mask_prob, targets, points):
    import ml_dtypes
    bf16 = ml_dtypes.bfloat16
    f32 = np.float32
    logits = np.ascontiguousarray(np.asarray(logits, dtype=f32))
    original_logits = np.ascontiguousarray(np.asarray(original_logits, dtype=f32))
    head_mask_prob = np.ascontiguousarray(np.asarray(head_mask_prob, dtype=f32))
    targets_f = np.asarray(targets).astype(f32)
    points = np.ascontiguousarray(np.asarray(points, dtype=f32))

    in_maps = []
    recon = []
    for b in range(B):
        hp = head_mask_prob[b]
        bmask = (hp > f32(0.3)) & (hp < f32(0.7))
        idx = np.flatnonzero(bmask)
        nb = idx.size
        assert nb <= NBP, f"boundary count {nb} exceeds {NBP}"
        pts = points[b][idx]
        order = np.argsort(pts[:, 0], kind="stable")
        pts_s = np.full((NBP, 3), f32(100.0))
        pts_s[:nb] = pts[order]
        lgs_s = np.zeros((NBP, 3), f32)
        lgs_s[:nb] = logits[b][idx][order]
        ptE = np.concatenate([
            np.full((PAD, 3), f32(-1000.0)), pts_s,
            np.full((PAD, 3), f32(2000.0))])
        lgE = np.concatenate([
            np.zeros((PAD, 3), f32), lgs_s, np.zeros((PAD, 3), f32)])

        xs = pts_s[:nb, 0]
        lo = np.searchsorted(xs, xs - f32(0.051), side="left")
        hi = np.searchsorted(xs, xs + f32(0.051), side="right")
        Wmax = max((np.arange(nb) - lo).max(), (hi - 1 - np.arange(nb)).max())
        assert Wmax <= PAD, f"rank window {Wmax} exceeds PAD={PAD}"

        recon.append(dict(nb=nb))

        lg = logits[b]
        m_full = ((lg[:, 2] > lg[:, 0]) & (lg[:, 2] > lg[:, 1])).astype(f32)

        for s in range(4):
            pi = pts_s[SLAB * s: SLAB * (s + 1)]
            a_i = pi.T.astype(bf16)
            b_i = (pi.T - a_i.astype(f32)).astype(bf16)
            nh = (f32(-0.5) * (pi * pi).sum(1, dtype=f32)).astype(f32)
            nh_a = nh.astype(bf16)
            nh_b = (nh - nh_a.astype(f32)).astype(bf16)
            pj = ptE[SLAB * s: SLAB * s + JW]
            a_j = pj.T.astype(bf16)
            b_j = (pj.T - a_j.astype(f32)).astype(bf16)
            rq = np.zeros((11, JW + SLAB), bf16)
            rq[0:3, 0:JW] = a_j
            rq[3:6, 0:JW] = a_j
            rq[6:9, 0:JW] = b_j
            rq[9:11, 0:JW] = np.ones((2, JW), bf16)
            rq[0:3, JW:] = a_i
            rq[3:6, JW:] = b_i
            rq[6:9, JW:] = a_i
            rq[9, JW:] = nh_a
            rq[10, JW:] = nh_b

            nrm_j = (pj * pj).sum(1, dtype=f32)
            mh_v = (f32(-0.5) * nrm_j).reshape(NCH, 128).T.astype(f32)
            mh_hi = mh_v.astype(bf16)
            mh_lo = (mh_v - mh_hi.astype(f32)).astype(bf16)
            lbw_v = lgE[SLAB * s: SLAB * s + JW].reshape(
                NCH, 128, 3).transpose(1, 0, 2).reshape(128, 33)

            q0 = QN * s
            hbl = np.zeros((128, 512), bf16)
            hbl[:, 448:481] = lbw_v.astype(bf16)
            hbl[:, 481:492] = mh_hi
            hbl[:, 492:503] = mh_lo
            hbl[:, 0:48] = logits[b][q0:q0 + QN].reshape(128, FQ, 3).transpose(
                0, 2, 1).reshape(128, 48).astype(bf16)
            hbl[:, 48:96] = original_logits[b][q0:q0 + QN].reshape(
                128, FQ, 3).transpose(0, 2, 1).reshape(128, 48).astype(bf16)
            hbl[:, 96:112] = targets_f[b][q0:q0 + QN].reshape(128, FQ).astype(bf16)
            hbl[:, 112:128] = bmask[q0:q0 + QN].astype(f32).reshape(
                128, FQ).astype(bf16)
            hbl[:, 128:192] = m_full.reshape(128, FN).astype(bf16)
            hbl[:, 192:384] = points[b].reshape(128, FN, 3).transpose(
                0, 2, 1).reshape(128, 192).astype(bf16)
            hbl[:, 384:400] = m_full[q0:q0 + QN].reshape(128, FQ).astype(bf16)
            hbl[:, 400:448] = points[b][q0:q0 + QN].reshape(128, FQ, 3).transpose(
                0, 2, 1).reshape(128, 48).astype(bf16)

            in_maps.append({"rq": rq, "hb": hbl})
    return in_maps, recon


def _postprocess(results, recon):
    totals = []
    for b in range(B):
        outs = results[4 * b:4 * b + 4]
        nb = recon[b]["nb"]
        S = [o["sums"][0].astype(np.float64) for o in outs]
        acc = np.concatenate([o["acc"] for o in outs], axis=1).astype(np.float64)
        corr = np.zeros((NBP, 3))
        for s in range(4):
            for i, ko in enumerate(SIGN_CHUNKS):
                lo, hi = _chunk_cols(ko)
                corr[SLAB * s + lo: SLAB * s + hi] += 0.5 * S[s][18 + 3 * i:21 + 3 * i]
        cnt = acc[0] + corr[:, 0]
        s1 = acc[1] + corr[:, 1]
        s2 = acc[2] + corr[:, 2]
        var = (s2 - s1 * s1 / np.maximum(cnt, 1.0)) / np.maximum(cnt - 1.0, 1.0)
        valid = (np.arange(NBP) < nb) & (cnt > 1.0)
        bm_sum = sum(Sx[15] for Sx in S)
        smooth = (var * valid).sum() / max(valid.sum(), 1.0) if bm_sum >= 5.0 else 0.0

        refinement = sum(Sx[10] for Sx in S) / N
        consistency = sum(Sx[11] + Sx[12] + Sx[13] for Sx in S) / (N * C)
        S0 = S[0]
        n = S0[0]
        ngt = sum(Sx[14] for Sx in S)
        nz = max(n, 1.0)
        Sx_ = S0[1:4]
        M2 = np.array([[S0[4], S0[7], S0[8]],
                       [S0[7], S0[5], S0[9]],
                       [S0[8], S0[9], S0[6]]])
        cen = Sx_ / nz
        cov = (M2 - np.outer(cen, Sx_) - np.outer(Sx_, cen)
               + n * np.outer(cen, cen)) / nz
        if n >= 10.0:
            ev = np.linalg.eigvalsh(cov)
            a = ev[2]
            shape = (ev[1] / (a + 1e-8) - 1.0) ** 2 + (ev[0] / (a + 1e-8) - 1.0) ** 2
        else:
            shape = 0.0
        Smd = sum(Sx[16] for Sx in S)
        Smd2 = sum(Sx[17] for Sx in S)
        mean_d = Smd / nz
        var_d = (Smd2 - 2.0 * mean_d * Smd + mean_d * mean_d * n) / max(n - 1.0, 1.0)
        max_d = max(Sx[33] for Sx in S)
        conn = var_d / (max_d + 1e-8) if n >= 5.0 else 0.0
        vol = (n - ngt) ** 2
        rel = abs(n - ngt) / max(ngt, 1.0)
        size = vol + 0.5 * rel if ngt > 0.0 else vol

        geometric = W_SHP * shape + W_SMO * smooth + W_SIZ * size + W_CNN * conn
        totals.append(W_REF * refinement + W_CON * consistency + geometric)
    return np.float32(np.mean(totals))


def run(trace=False, **inputs):
    nc = _get_nc()
    in_maps, recon = _prep_inputs(**inputs)
    res = run_bass_kernel_spmd(nc, in_maps, core_ids=list(range(NCORES)),
                               trace=trace)
    out = _postprocess(res.results, recon)
    return out, res


def kernel(logits, original_logits, head_mask_prob, targets, points):
    out, _ = run(logits=logits, original_logits=original_logits,
                 head_mask_prob=head_mask_prob, targets=targets, points=points)
    return out


# revision 6
# speedup vs baseline: 1.0606x; 1.0606x over previous
"""Trainium2 Bass kernel for nn_CabbageHeadRefinementLoss — pruned redesign.

Self-contained: accepts FULL inputs, shards across 8 NeuronCores internally,
returns the FULL (scalar) output.

Strategy (v2 — x-sort pruned ball query):
  - Boundary points (~3250 of 8192) are compacted AND SORTED BY X on host,
    padded to NBP=3584.  Each core owns an i-slab of 896 sorted ranks; only
    j's within PAD=256 ranks of the slab can be within R=0.05 in x (host
    asserts the true rank window <= PAD), so each core processes an 11-chunk
    j-window of 1408 ranks instead of all 3584: 4480 matmul columns/core
    instead of 25088 (5.6x less PE work).
  - Per j-chunk k (128 j's), i-columns C_k = [128k-512, 128k+128) ∩ [0,896):
    mm1 (bf16 hi/lo split, K=11) -> d2 PSUM; threshold alternates DVE is_gt
    (even k) / ACT Sign with half-weights (odd k, corrected on host); mm2
    accumulates [1, p2, p2^2] per 128-col block with exact start/stop flags.
  - No PE warm-up: total matmul work (~9k cycles) is below the HAM unthrottle
    threshold, so the loop targets the steady 1.2 GHz p-state.
  - O(N) terms: refinement/consistency/target-counts sharded in quarters
    across the 4 cores of a sample (bf16 inputs); pred-head mask computed on
    host (must be exact); moments + center + distance pass replicated.
    sqrt(s) computed as exp(0.5*ln(s)) so only ONE ACT table set (id 6:
    exp/ln/sign/copy) is ever loaded.
  - All inputs packed into 3 per-partition-contiguous DMA blobs; outputs are
    row-contiguous (no [128,1]-shaped DMAs -> no completion-counter straggler).
  - Host combines: per-row variance, 3x3 eigendecomposition, gates, total.
"""

import numpy as np

try:
    import concourse.bass as bass
except ImportError:  # fallback for environments without NIX_PYTHONPATH
    import sys
    sys.path.insert(0, "/opt/trn_rl_repo")
    import concourse.bass as bass

import concourse.mybir as mybir
import concourse.tile as tile
from concourse import bacc
from concourse.bass_utils import run_bass_kernel_spmd

F32 = mybir.dt.float32
BF16 = mybir.dt.bfloat16
ALU = mybir.AluOpType
ACTF = mybir.ActivationFunctionType

B, N, C = 2, 8192, 3
R2 = np.float32(0.05) * np.float32(0.05)
W_REF, W_CON, W_BND = 0.3, 0.2, 2.0
W_SHP, W_SMO, W_SIZ, W_CNN = 0.5, 0.3, 0.8, 0.6

NBP = 3584
SLAB = 896          # i-ranks per core
PAD = 256           # j rank-window halo (host asserts data fits)
JW = SLAB + 2 * PAD  # 1408 j-window
NCH = JW // 128      # 11 j-chunks
NBLK = SLAB // 128   # 7 acc column blocks
QN = N // 4          # 2048 per-core quarter for sharded O(N) terms
FQ = QN // 128       # 16
FN = N // 128        # 64
NCORES = 8

SIGN_CHUNKS = (1, 3, 5, 7, 9)   # chunks thresholded via ACT Sign (half-weights)

# sums row layout (host side):
#   [0]=n  [1:4]=Smx  [4:10]=M2(xx,yy,zz,xy,xz,yz)          <- ones-matmul #1
#   [10]=nllw_q [11:14]=cons_c [14]=ngt_q [15]=bm_sum_q
#   [16]=Smd_q [17]=Smd2_q [18:33]=vodd(k in SIGN_CHUNKS)   <- ones-matmul #2
#   [33]=maxd_q

_NC_CACHE = None


def _chunk_cols(k):
    return max(0, 128 * k - 2 * PAD), min(SLAB, 128 * k + 128)


def _blk_range(k):
    return max(0, k - 4), min(NBLK - 1, k)


def _build_nc():
    nc = bacc.Bacc("TRN2", target_bir_lowering=False, debug=False,
                   enable_asserts=False)

    rq_d = nc.dram_tensor("rq", [11, JW + SLAB], BF16, kind="ExternalInput").ap()
    hb_d = nc.dram_tensor("hb", [128, 512], BF16, kind="ExternalInput").ap()
    acc_d = nc.dram_tensor("acc", [3, SLAB], F32, kind="ExternalOutput").ap()
    sums_d = nc.dram_tensor("sums", [1, 64], F32, kind="ExternalOutput").ap()

    thr = float(-R2 / 2.0)

    with tile.TileContext(nc) as tc:
        with (
            tc.tile_pool(name="const", bufs=1) as const,
            tc.tile_pool(name="work", bufs=8) as work,
            tc.tile_pool(name="tp", bufs=4) as tp,
            tc.tile_pool(name="psD", bufs=2, space="PSUM") as psD,
            tc.tile_pool(name="psN", bufs=1, space="PSUM") as psN,
            tc.tile_pool(name="psS", bufs=1, space="PSUM") as psS,
            tc.tile_pool(name="psA", bufs=1, space="PSUM") as psA,
        ):
            # Load ACT set 6 (exp+ln+sign+copy) once, up front: the greedy
            # table-load pass would otherwise thrash between sets 0 and 5
            # (1.28us per reload on the Scalar critical path).
            _li = mybir.InstLoadActFuncSet(
                name=nc.get_next_instruction_name(), ins=[], outs=[],
                act_func_set_id=6)
            nc.scalar.add_instruction(_li)

            # ---------- input DMAs ----------
            RQ = const.tile([11, JW + SLAB], BF16)
            nc.sync.dma_start(RQ[:], rq_d[:])
            H = const.tile([128, 512], BF16)
            nc.gpsimd.dma_start(H[:], hb_d[:])

            RB = RQ[:, 0:JW]
            QB = RQ[:, JW:JW + SLAB]
            lbw = H[:, 448:481].rearrange("p (k c) -> p k c", c=3)
            with tc.high_priority():
                mh_t = const.tile([128, 11], F32)
                nc.vector.tensor_add(mh_t[:], H[:, 481:492], H[:, 492:503])
                biasj_t = const.tile([128, 11], F32)
                nc.vector.tensor_scalar(biasj_t[:], mh_t[:], float(R2) / 2.0,
                                        None, op0=ALU.add)
            mh = mh_t
            biasj = biasj_t
            lgq = H[:, 0:48].rearrange("p (c f) -> p c f", c=3)
            loq = H[:, 48:96].rearrange("p (c f) -> p c f", c=3)
            tgq = H[:, 96:112]
            bmq = H[:, 112:128]
            m_f = H[:, 128:192]
            ptT = H[:, 192:384].rearrange("p (c f) -> p c f", c=3)
            m_q = H[:, 384:400]
            ptq = H[:, 400:448].rearrange("p (c f) -> p c f", c=3)

            stA = const.tile([128, 10], F32)   # n, Smx, M2
            st2 = const.tile([128, 23], F32)   # nllw, cons, ge2, bm_sum, Smd,
            junk = const.tile([128, FN], F32)  # Smd2, vodd
            junk2 = const.tile([128, FN], F32)

            acc_ps = psA.tile([3, SLAB], F32, tag="accp", name="acc_ps")
            with tc.high_priority():
                nc.vector.memset(acc_ps[:], 0.0)

            def d2t(k):
                lo, hi = _chunk_cols(k)
                w = hi - lo
                lhsT = RB[:, 128 * k:128 * (k + 1)]
                with tc.high_priority():
                    if k >= 8:
                        d2 = psN.tile([128, 384], F32, tag="d2n", name=f"d2_{k}")
                    else:
                        d2 = psD.tile([128, 640], F32, tag="d2", name=f"d2_{k}")
                    for c0 in range(0, w, 512):
                        c1 = min(c0 + 512, w)
                        nc.tensor.matmul(d2[:, c0:c1], lhsT,
                                         QB[:, lo + c0:lo + c1],
                                         start=True, stop=True)
                    T = tp.tile([128, 640], BF16, tag="T", name=f"T_{k}")
                    if k not in SIGN_CHUNKS:
                        nc.vector.tensor_scalar(T[:, 0:w], d2[:, 0:w],
                                                mh[:, k:k + 1], thr,
                                                op0=ALU.add, op1=ALU.is_gt)
                    else:
                        nc.scalar.activation(T[:, 0:w], d2[:, 0:w], ACTF.Sign,
                                             bias=biasj[:, k:k + 1], scale=1.0)
                return T

            def accm(k, T):
                lo, hi = _chunk_cols(k)
                w = hi - lo
                V = Vh if k in SIGN_CHUNKS else Vb
                with tc.high_priority():
                    # accumulating matmul (PSUM pre-zeroed by the memset)
                    for c0 in range(0, w, 512):
                        c1 = min(c0 + 512, w)
                        nc.tensor.matmul(acc_ps[:, lo + c0:lo + c1],
                                         V[:, k, :], T[:, c0:c1],
                                         start=False, stop=False,
                                         skip_group_check=True)

            def chunk(k):
                accm(k, d2t(k))

            # ---------- chunks 0-1: thresholds first, V build, then mm2 ----------
            T0 = d2t(0)
            T1 = d2t(1)

            # ---------- boundary p2 / V (dep: H) ----------
            with tc.high_priority():
                ELB = work.tile([128, 11, 3], F32)
                nc.scalar.activation(ELB[:], lbw, ACTF.Exp)
                sb = work.tile([128, 11], F32)
                nc.vector.tensor_add(sb[:], ELB[:, :, 0], ELB[:, :, 1])
                sb2 = work.tile([128, 11], F32)
                nc.vector.tensor_add(sb2[:], sb[:], ELB[:, :, 2])
                rb = work.tile([128, 11], F32)
                nc.vector.reciprocal(rb[:], sb2[:])
                p2 = work.tile([128, 11], F32)
                nc.vector.tensor_mul(p2[:], ELB[:, :, 2], rb[:])
                Vb = const.tile([128, 11, 3], BF16)
                nc.vector.memset(Vb[:, :, 0:1], 1.0)
                nc.vector.tensor_copy(Vb[:, :, 1], p2[:])
                nc.vector.tensor_mul(Vb[:, :, 2], p2[:], p2[:])
                Vh = const.tile([128, 11, 3], BF16)
                nc.vector.tensor_scalar(Vh[:], Vb[:], 0.5, None, op0=ALU.mult)

            accm(0, T0)
            accm(1, T1)


            # ---------- O(N) stage A: count + moments (dep: H) ----------
            with tc.tile_wait_until(0.004):
                nc.vector.tensor_reduce(stA[:, 0:1], m_f,
                                        axis=mybir.AxisListType.X, op=ALU.add)
                mx = []
                for c in range(3):
                    mxc = const.tile([128, FN], F32, tag=f"mx{c}", name=f"mx{c}")
                    nc.vector.scalar_tensor_tensor(
                        out=mxc[:], in0=m_f, scalar=0.0, in1=ptT[:, c, :],
                        op0=ALU.add, op1=ALU.mult, accum_out=stA[:, 1 + c:2 + c])
                    mx.append(mxc)
                pairs = [(0, 0), (1, 1), (2, 2), (0, 1), (0, 2), (1, 2)]
                for kk, (a, bb) in enumerate(pairs):
                    nc.vector.scalar_tensor_tensor(
                        out=junk2[:], in0=mx[a][:], scalar=0.0, in1=ptT[:, bb, :],
                        op0=ALU.add, op1=ALU.mult, accum_out=stA[:, 4 + kk:5 + kk])

            chunk(2)
            chunk(3)

            # ---------- center chain ----------
            ones1 = const.tile([128, 1], F32)
            nc.vector.memset(ones1[:], 1.0)
            sums1 = psS.tile([1, 10], F32, tag="s1", name="sums1")
            nc.tensor.matmul(sums1[:], ones1[:], stA[:], start=True, stop=True)
            nz = work.tile([1, 1], F32)
            nc.vector.tensor_scalar(nz[:], sums1[0:1, 0:1], 1.0, None, op0=ALU.max)
            rcp = work.tile([1, 1], F32)
            nc.vector.reciprocal(rcp[:], nz[:])
            cen = work.tile([1, 3], F32)
            nc.vector.tensor_scalar(cen[:], sums1[0:1, 1:4], rcp[:], None,
                                    op0=ALU.mult)
            sums_sb = const.tile([1, 64], F32)
            nc.vector.tensor_copy(sums_sb[:, 0:10], sums1[:])
            ones2 = const.tile([1, 128], F32)
            nc.vector.memset(ones2[:], 1.0)
            cbp = psS.tile([128, 3], F32, tag="s1", name="cbp")
            nc.tensor.matmul(cbp[:], ones2[:], cen[:], start=True, stop=True)
            cb = const.tile([128, 3], F32)
            nc.vector.tensor_copy(cb[:], cbp[:])

            chunk(4)

            # ---------- O(N) stage B: softmax / consistency / nll (quarter) ----------
            tc.tile_set_cur_wait(0.005)
            EL = work.tile([128, 3, FQ], F32)
            nc.scalar.activation(EL[:], lgq, ACTF.Exp)
            sl = work.tile([128, FQ], F32)
            nc.vector.tensor_add(sl[:], EL[:, 0, :], EL[:, 1, :])
            sl2 = work.tile([128, FQ], F32)
            nc.vector.tensor_add(sl2[:], sl[:], EL[:, 2, :])
            rl = work.tile([128, FQ], F32)
            nc.vector.reciprocal(rl[:], sl2[:])
            EO = work.tile([128, 3, FQ], F32)
            nc.scalar.activation(EO[:], loq, ACTF.Exp)
            so = work.tile([128, FQ], F32)
            nc.gpsimd.tensor_add(so[:], EO[:, 0, :], EO[:, 1, :])
            so2 = work.tile([128, FQ], F32)
            nc.gpsimd.tensor_add(so2[:], so[:], EO[:, 2, :])
            ro = work.tile([128, FQ], F32)
            nc.vector.reciprocal(ro[:], so2[:])
            lnS = work.tile([128, FQ], F32)
            nc.scalar.activation(lnS[:], sl2[:], ACTF.Ln)

            tc.tile_set_cur_wait(0)
            chunk(5)
            tc.tile_set_cur_wait(0.005)

            for c in range(3):
                pc = work.tile([128, FQ], F32, tag="pc", name="pc")
                nc.vector.tensor_mul(pc[:], EL[:, c, :], rl[:])
                qc = work.tile([128, FQ], F32, tag="qc", name="qc")
                nc.gpsimd.tensor_mul(qc[:], EO[:, c, :], ro[:])
                dc = work.tile([128, FQ], F32, tag="dc", name="dc")
                nc.gpsimd.tensor_sub(dc[:], pc[:], qc[:])
                nc.vector.scalar_tensor_tensor(
                    out=junk2[:, 0:FQ], in0=dc[:], scalar=0.0, in1=dc[:],
                    op0=ALU.add, op1=ALU.mult, accum_out=st2[:, 1 + c:2 + c])

            lt = None
            for c in range(3):
                mc = work.tile([128, FQ], F32, tag=f"mc{c}", name=f"mc{c}")
                nc.vector.tensor_scalar(mc[:], tgq, float(c), None,
                                        op0=ALU.is_equal)
                lm = work.tile([128, FQ], F32, tag="lm", name="lm")
                nc.gpsimd.tensor_mul(lm[:], lgq[:, c, :], mc[:])
                if lt is None:
                    lt = lm
                else:
                    lt2 = work.tile([128, FQ], F32, tag="lt2", name="lt2")
                    nc.gpsimd.tensor_add(lt2[:], lt[:], lm[:])
                    lt = lt2
                if c == 2:
                    nc.vector.tensor_reduce(st2[:, 4:5], mc[:],
                                            axis=mybir.AxisListType.X, op=ALU.add)
            nll = work.tile([128, FQ], F32)
            nc.vector.tensor_sub(nll[:], lnS[:], lt[:])
            nc.vector.scalar_tensor_tensor(
                out=junk2[:, 0:FQ], in0=bmq, scalar=1.0, in1=nll[:],
                op0=ALU.add, op1=ALU.mult, accum_out=st2[:, 0:1])
            nc.vector.tensor_reduce(st2[:, 5:6], bmq, axis=mybir.AxisListType.X,
                                    op=ALU.add)
            tc.tile_set_cur_wait(0)

            chunk(6)

            # ---------- distance pass (replicated full sample) ----------
            dx = work.tile([128, FQ], F32)
            nc.vector.tensor_scalar(dx[:], ptq[:, 0, :], cb[:, 0:1], None,
                                    op0=ALU.subtract)
            dy = work.tile([128, FQ], F32)
            nc.vector.tensor_scalar(dy[:], ptq[:, 1, :], cb[:, 1:2], None,
                                    op0=ALU.subtract)
            dz = work.tile([128, FQ], F32)
            nc.vector.tensor_scalar(dz[:], ptq[:, 2, :], cb[:, 2:3], None,
                                    op0=ALU.subtract)
            s0 = work.tile([128, FQ], F32)
            nc.gpsimd.tensor_mul(s0[:], dx[:], dx[:])
            s1t = work.tile([128, FQ], F32)
            nc.vector.tensor_mul(s1t[:], dy[:], dy[:])
            s2t = work.tile([128, FQ], F32)
            nc.gpsimd.tensor_add(s2t[:], s0[:], s1t[:])
            s3t = work.tile([128, FQ], F32)
            nc.gpsimd.tensor_mul(s3t[:], dz[:], dz[:])
            s4t = work.tile([128, FQ], F32)
            nc.vector.tensor_add(s4t[:], s2t[:], s3t[:])
            eps12 = const.tile([128, 1], F32)
            nc.vector.memset(eps12[:], 1e-12)
            ls = work.tile([128, FQ], F32)
            nc.scalar.activation(ls[:], s4t[:], ACTF.Ln, bias=eps12[:, 0:1])
            dd = work.tile([128, FQ], F32)
            nc.scalar.activation(dd[:], ls[:], ACTF.Exp, bias=0.0, scale=0.5)
            md = work.tile([128, FQ], F32)
            nc.gpsimd.tensor_mul(md[:], m_q, dd[:])
            nc.vector.tensor_reduce(st2[:, 6:7], md[:], axis=mybir.AxisListType.X,
                                    op=ALU.add)
            nc.vector.scalar_tensor_tensor(
                out=junk2[:, 0:FQ], in0=md[:], scalar=0.0, in1=dd[:],
                op0=ALU.add, op1=ALU.mult, accum_out=st2[:, 7:8])
            maxsc = work.tile([1, 1], F32)
            nc.gpsimd.tensor_reduce(maxsc[:], md[:], axis=mybir.AxisListType.XYZWC,
                                    op=ALU.max)
            nc.vector.tensor_copy(sums_sb[:, 33:34], maxsc[:])

            chunk(7)

            # vodd copies for host Sign correction
            for i, ko in enumerate(SIGN_CHUNKS):
                nc.gpsimd.tensor_copy(st2[:, 8 + 3 * i:11 + 3 * i], Vb[:, ko, :])

            chunk(8)

            # acc cols [0:640] complete after chunk 8 (contributors k<=8)
            acc_sb = const.tile([3, SLAB], F32)
            nc.scalar.copy(acc_sb[:, 0:640], acc_ps[:, 0:640])
            nc.sync.dma_start(acc_d[:, 0:640], acc_sb[:, 0:640])

            chunk(9)

            sums2 = psS.tile([1, 23], F32, tag="s1", name="sums2")
            nc.tensor.matmul(sums2[:], ones1[:], st2[:], start=True, stop=True)
            nc.vector.tensor_copy(sums_sb[:, 10:33], sums2[:])
            nc.sync.dma_start(sums_d[:], sums_sb[:])
            nc.vector.tensor_copy(acc_sb[:, 640:768], acc_ps[:, 640:768])

            chunk(10)

            nc.vector.tensor_copy(acc_sb[:, 768:SLAB], acc_ps[:, 768:SLAB])
            nc.sync.dma_start(acc_d[:, 640:SLAB], acc_sb[:, 640:SLAB])

    nc.compile()
    return nc


def _get_nc():
    global _NC_CACHE
    if _NC_CACHE is None:
        _NC_CACHE = _build_nc()
    return _NC_CACHE


def _prep_inputs(logits, original_logits, head_mask_prob, targets, points):
    import ml_dtypes
    bf16 = ml_dtypes.bfloat16
    f32 = np.float32
    logits = np.ascontiguousarray(np.asarray(logits, dtype=f32))
    original_logits = np.ascontiguousarray(np.asarray(original_logits, dtype=f32))
    head_mask_prob = np.ascontiguousarray(np.asarray(head_mask_prob, dtype=f32))
    targets_f = np.asarray(targets).astype(f32)
    points = np.ascontiguousarray(np.asarray(points, dtype=f32))

    in_maps = []
    recon = []
    for b in range(B):
        hp = head_mask_prob[b]
        bmask = (hp > f32(0.3)) & (hp < f32(0.7))
        idx = np.flatnonzero(bmask)
        nb = idx.size
        assert nb <= NBP, f"boundary count {nb} exceeds {NBP}"
        pts = points[b][idx]
        order = np.argsort(pts[:, 0], kind="stable")
        pts_s = np.full((NBP, 3), f32(100.0))
        pts_s[:nb] = pts[order]
        lgs_s = np.zeros((NBP, 3), f32)
        lgs_s[:nb] = logits[b][idx][order]
        ptE = np.concatenate([
            np.full((PAD, 3), f32(-1000.0)), pts_s,
            np.full((PAD, 3), f32(2000.0))])
        lgE = np.concatenate([
            np.zeros((PAD, 3), f32), lgs_s, np.zeros((PAD, 3), f32)])

        xs = pts_s[:nb, 0]
        lo = np.searchsorted(xs, xs - f32(0.051), side="left")
        hi = np.searchsorted(xs, xs + f32(0.051), side="right")
        Wmax = max((np.arange(nb) - lo).max(), (hi - 1 - np.arange(nb)).max())
        assert Wmax <= PAD, f"rank window {Wmax} exceeds PAD={PAD}"

        recon.append(dict(nb=nb))

        lg = logits[b]
        m_full = ((lg[:, 2] > lg[:, 0]) & (lg[:, 2] > lg[:, 1])).astype(f32)

        for s in range(4):
            pi = pts_s[SLAB * s: SLAB * (s + 1)]
            a_i = pi.T.astype(bf16)
            b_i = (pi.T - a_i.astype(f32)).astype(bf16)
            nh = (f32(-0.5) * (pi * pi).sum(1, dtype=f32)).astype(f32)
            nh_a = nh.astype(bf16)
            nh_b = (nh - nh_a.astype(f32)).astype(bf16)
            pj = ptE[SLAB * s: SLAB * s + JW]
            a_j = pj.T.astype(bf16)
            b_j = (pj.T - a_j.astype(f32)).astype(bf16)
            rq = np.zeros((11, JW + SLAB), bf16)
            rq[0:3, 0:JW] = a_j
            rq[3:6, 0:JW] = a_j
            rq[6:9, 0:JW] = b_j
            rq[9:11, 0:JW] = np.ones((2, JW), bf16)
            rq[0:3, JW:] = a_i
            rq[3:6, JW:] = b_i
            rq[6:9, JW:] = a_i
            rq[9, JW:] = nh_a
            rq[10, JW:] = nh_b

            nrm_j = (pj * pj).sum(1, dtype=f32)
            mh_v = (f32(-0.5) * nrm_j).reshape(NCH, 128).T.astype(f32)
            mh_hi = mh_v.astype(bf16)
            mh_lo = (mh_v - mh_hi.astype(f32)).astype(bf16)
            lbw_v = lgE[SLAB * s: SLAB * s + JW].reshape(
                NCH, 128, 3).transpose(1, 0, 2).reshape(128, 33)

            q0 = QN * s
            hbl = np.zeros((128, 512), bf16)
            hbl[:, 448:481] = lbw_v.astype(bf16)
            hbl[:, 481:492] = mh_hi
            hbl[:, 492:503] = mh_lo
            hbl[:, 0:48] = logits[b][q0:q0 + QN].reshape(128, FQ, 3).transpose(
                0, 2, 1).reshape(128, 48).astype(bf16)
            hbl[:, 48:96] = original_logits[b][q0:q0 + QN].reshape(
                128, FQ, 3).transpose(0, 2, 1).reshape(128, 48).astype(bf16)
            hbl[:, 96:112] = targets_f[b][q0:q0 + QN].reshape(128, FQ).astype(bf16)
            hbl[:, 112:128] = bmask[q0:q0 + QN].astype(f32).reshape(
                128, FQ).astype(bf16)
            hbl[:, 128:192] = m_full.reshape(128, FN).astype(bf16)
            hbl[:, 192:384] = points[b].reshape(128, FN, 3).transpose(
                0, 2, 1).reshape(128, 192).astype(bf16)
            hbl[:, 384:400] = m_full[q0:q0 + QN].reshape(128, FQ).astype(bf16)
            hbl[:, 400:448] = points[b][q0:q0 + QN].reshape(128, FQ, 3).transpose(
                0, 2, 1).reshape(128, 48).astype(bf16)

            in_maps.append({"rq": rq, "hb": hbl})
    return in_maps, recon


def _postprocess(results, recon):
    totals = []
    for b in range(B):
        outs = results[4 * b:4 * b + 4]
        nb = recon[b]["nb"]
        S = [o["sums"][0].astype(np.float64) for o in outs]
        acc = np.concatenate([o["acc"] for o in outs], axis=1).astype(np.float64)
        corr = np.zeros((NBP, 3))
        for s in range(4):
            for i, ko in enumerate(SIGN_CHUNKS):
                lo, hi = _chunk_cols(ko)
                corr[SLAB * s + lo: SLAB * s + hi] += 0.5 * S[s][18 + 3 * i:21 + 3 * i]
        cnt = acc[0] + corr[:, 0]
        s1 = acc[1] + corr[:, 1]
        s2 = acc[2] + corr[:, 2]
        var = (s2 - s1 * s1 / np.maximum(cnt, 1.0)) / np.maximum(cnt - 1.0, 1.0)
        valid = (np.arange(NBP) < nb) & (cnt > 1.0)
        bm_sum = sum(Sx[15] for Sx in S)
        smooth = (var * valid).sum() / max(valid.sum(), 1.0) if bm_sum >= 5.0 else 0.0

        refinement = sum(Sx[10] for Sx in S) / N
        consistency = sum(Sx[11] + Sx[12] + Sx[13] for Sx in S) / (N * C)
        S0 = S[0]
        n = S0[0]
        ngt = sum(Sx[14] for Sx in S)
        nz = max(n, 1.0)
        Sx_ = S0[1:4]
        M2 = np.array([[S0[4], S0[7], S0[8]],
                       [S0[7], S0[5], S0[9]],
                       [S0[8], S0[9], S0[6]]])
        cen = Sx_ / nz
        cov = (M2 - np.outer(cen, Sx_) - np.outer(Sx_, cen)
               + n * np.outer(cen, cen)) / nz
        if n >= 10.0:
            ev = np.linalg.eigvalsh(cov)
            a = ev[2]
            shape = (ev[1] / (a + 1e-8) - 1.0) ** 2 + (ev[0] / (a + 1e-8) - 1.0) ** 2
        else:
            shape = 0.0
        Smd = sum(Sx[16] for Sx in S)
        Smd2 = sum(Sx[17] for Sx in S)
        mean_d = Smd / nz
        var_d = (Smd2 - 2.0 * mean_d * Smd + mean_d * mean_d * n) / max(n - 1.0, 1.0)
        max_d = max(Sx[33] for Sx in S)
        conn = var_d / (max_d + 1e-8) if n >= 5.0 else 0.0
        vol = (n - ngt) ** 2
        rel = abs(n - ngt) / max(ngt, 1.0)
        size = vol + 0.5 * rel if ngt > 0.0 else vol

        geometric = W_SHP * shape + W_SMO * smooth + W_SIZ * size + W_CNN * conn
        totals.append(W_REF * refinement + W_CON * consistency + geometric)
    return np.float32(np.mean(totals))


def run(trace=False, **inputs):
    nc = _get_nc()
    in_maps, recon = _prep_inputs(**inputs)
    res = run_bass_kernel_spmd(nc, in_maps, core_ids=list(range(NCORES)),
                               trace=trace)
    out = _postprocess(res.results, recon)
    return out, res


def kernel(logits, original_logits, head_mask_prob, targets, points):
    out, _ = run(logits=logits, original_logits=original_logits,
                 head_mask_prob=head_mask_prob, targets=targets, points=points)
    return out
